# revision 13
# baseline (speedup 1.0000x reference)
"""Multi-head attention with RoPE (B=4, N=2048, C=1024, H=16, d=64) on 8
Trainium2 NeuronCores.

Sharding: tensor-parallel over heads — each core computes 2 of the 16 heads
(Wq/Wkv sharded column-wise, Wout row-wise). Each core returns a partial
yT = (out_h @ Wout_h).T over the full batch; the host sums the 8 partials.

Per-core kernel (all matmuls bf16, fp32 PSUM accumulation), software-pipelined
so ScalarE streams exp() continuously while the PE array stays dense:
  - Projections run W-stationary one 512-token chunk at a time (8 contraction
    matmuls into one dedicated PSUM bank pair); chunks are emitted as filler
    work at attention chunk boundaries. RoPE is applied on evacuation
    (rotate-half via partition-swapped SBUF copy, sign folded into the host
    sin table); all PSUM evacuations of projections run on VectorE so ScalarE
    does nothing but exp.
  - v is transposed token-major via DRAM bounce + XBAR DMA transpose with
    interleaved ones columns so PV (M=65) also yields softmax denominators.
  - Attention per 512-query chunk: S^T tiles = k.T@q, two heads per PE pass
    via row-group tile_position; exp on ScalarE from PSUM (scale=1/8 folded;
    |S/8| < 3 so no max subtraction); PV accumulates O^T + denominators.
  - The chunk boundary is software-pipelined: output projection of chunk i is
    interleaved with projection-chunk fillers and the first 7 QK+exp groups
    of chunk i+1, so neither PE nor ScalarE idles; PV of chunk i+1 catches up
    two-per-slot once its PSUM accumulators free.
"""

import numpy as np
import ml_dtypes
from contextlib import ExitStack

import concourse.bass as bass
import concourse.tile as tile
from concourse import bacc, mybir
from concourse.bass_utils import run_bass_kernel_spmd

P = 128
B, NSEQ, C = 4, 2048, 1024
H, D = 16, 64
NTOK = B * NSEQ
KO = C // P
QC = 512
NKT = NSEQ // P
NQC = NSEQ // QC
FC = C // P
VW = 160  # vtok row width: [v_h0 | 1 | v_h1 | 1 | pad] (32-multiple for XBAR)
BF = mybir.dt.bfloat16
F32 = mybir.dt.float32

PRE = 7  # QK+exp groups of chunk i+1 emitted during chunk i's boundary


def _build():
    nc = bacc.Bacc("TRN2", target_bir_lowering=False, debug=False)

    xT = nc.dram_tensor("xT", [C, NTOK], BF, kind="ExternalInput").ap()
    wq = nc.dram_tensor("wq", [C, P], BF, kind="ExternalInput").ap()
    wk = nc.dram_tensor("wk", [C, P], BF, kind="ExternalInput").ap()
    wv = nc.dram_tensor("wv", [C, P], BF, kind="ExternalInput").ap()
    wout = nc.dram_tensor("wout", [P, C], BF, kind="ExternalInput").ap()
    cos2 = nc.dram_tensor("cos2", [P, NSEQ], F32, kind="ExternalInput").ap()
    sin2s = nc.dram_tensor("sin2s", [P, NSEQ], F32, kind="ExternalInput").ap()
    yT = nc.dram_tensor("yT", [C, NTOK], F32, kind="ExternalOutput").ap()

    with ExitStack() as ctx:
        tc = ctx.enter_context(tile.TileContext(nc))
        consts = ctx.enter_context(tc.tile_pool(name="consts", bufs=1))
        xpool = ctx.enter_context(tc.tile_pool(name="xpool", bufs=2))
        qkpool = ctx.enter_context(tc.tile_pool(name="qkpool", bufs=2))
        vpool = ctx.enter_context(tc.tile_pool(name="vpool", bufs=2))
        rope = ctx.enter_context(tc.tile_pool(name="rope", bufs=2))
        pexp_pool = ctx.enter_context(tc.tile_pool(name="pexp", bufs=10))
        onorm_pool = ctx.enter_context(tc.tile_pool(name="onorm", bufs=3))
        ytmp_pool = ctx.enter_context(tc.tile_pool(name="ytmp", bufs=4))
        small = ctx.enter_context(tc.tile_pool(name="small", bufs=2))
        dram = ctx.enter_context(tc.tile_pool(name="dram", bufs=2, space="DRAM"))
        ps_s = ctx.enter_context(tc.tile_pool(name="ps_s", bufs=2, space="PSUM"))
        ps_po = ctx.enter_context(tc.tile_pool(name="ps_po", bufs=2, space="PSUM"))
        ps_pj = ctx.enter_context(tc.tile_pool(name="ps_pj", bufs=2, space="PSUM"))

        # ---- constants ----
        wq_sb = consts.tile([P, KO, P], BF, tag="wq")
        wk_sb = consts.tile([P, KO, P], BF, tag="wk")
        wv_sb = consts.tile([P, KO, P], BF, tag="wv")
        wout_sb = consts.tile([P, FC, P], BF, tag="wout")
        cos_sb = consts.tile([P, NSEQ], F32, tag="cos")
        sin_sb = consts.tile([P, NSEQ], F32, tag="sin")
        nc.sync.dma_start(wq_sb[:], wq.rearrange("(ko p) f -> p ko f", p=P))
        nc.sync.dma_start(wk_sb[:], wk.rearrange("(ko p) f -> p ko f", p=P))
        nc.sync.dma_start(wv_sb[:], wv.rearrange("(ko p) f -> p ko f", p=P))
        nc.sync.dma_start(wout_sb[:], wout.rearrange("r (fc f) -> r fc f", f=P))
        nc.sync.dma_start(cos_sb[:], cos2)
        nc.sync.dma_start(sin_sb[:], sin2s)
        ones_row = consts.tile([1, NSEQ], BF, tag="ones_row")
        nc.vector.memset(ones_row[:], 1.0)
        ones_blk = consts.tile([32, NSEQ], BF, tag="ones_blk")
        nc.vector.memset(ones_blk[:], 1.0)
        vbounces = []
        for i in range(2):
            vb = dram.tile([VW, NSEQ], BF, tag="vbounce", name=f"vb{i}")
            nc.sync.dma_start(vb[2 * D + 2 : VW, :], ones_blk[: VW - 2 * D - 2, :])
            vbounces.append(vb)

        def emit_load(b):
            t0 = b * NSEQ
            xb = xpool.tile([P, KO, NSEQ], BF, tag="xb", name="xb")
            xr = xT[:, t0 : t0 + NSEQ].rearrange("(ko p) t -> p ko t", p=P)
            # token-chunk granular so the first projection chunk only waits
            # on 1/4 of the batch's x slab
            for t4 in range(4):
                tsl = slice(t4 * QC, (t4 + 1) * QC)
                for ko in range(KO):
                    nc.sync.dma_start(xb[:, ko, tsl], xr[:, ko, tsl])
            qTt = qkpool.tile([P, NSEQ], BF, tag="qT", name="qT")
            kTt = qkpool.tile([P, NSEQ], BF, tag="kT", name="kT")
            vTt = qkpool.tile([P, NSEQ], BF, tag="vT", name="vT")
            vtok = vpool.tile([P, NKT, VW], BF, tag="vtok", name="vtok")
            return dict(xb=xb, qT=qTt, kT=kTt, vT=vTt, vtok=vtok, b=b)

        def emit_proj_chunk(st, f, t4):
            """One 512-token chunk of projection f (0=q, 1=k, 2=v):
            8 contraction matmuls into one PSUM bank + evacuation tail.
            All evacuations on VectorE (ScalarE is reserved for exp)."""
            w_sb, dst = [(wq_sb, st["qT"]), (wk_sb, st["kT"]), (wv_sb, st["vT"])][f]
            tsl = slice(t4 * QC, (t4 + 1) * QC)
            pj = ps_pj.tile([P, QC], F32, tag="pj", name="pj")
            for ko in range(KO):
                nc.tensor.matmul(
                    pj[:],
                    w_sb[:, ko, :],
                    st["xb"][:, ko, tsl],
                    start=(ko == 0),
                    stop=(ko == KO - 1),
                    skip_group_check=True,
                )
            if f == 2:
                nc.vector.tensor_copy(st["vT"][:, tsl], pj[:])
            else:
                raw = rope.tile([P, QC], F32, tag="raw", name="raw")
                swp = rope.tile([P, QC], F32, tag="swp", name="swp")
                qcs = rope.tile([P, QC], F32, tag="qcs", name="qcs")
                qss = rope.tile([P, QC], F32, tag="qss", name="qss")
                nc.vector.tensor_copy(raw[:], pj[:])
                for blk in range(4):
                    src = (blk ^ 1) * 32
                    nc.sync.dma_start(
                        swp[blk * 32 : blk * 32 + 32, :], raw[src : src + 32, :]
                    )
                nc.vector.tensor_mul(qcs[:], raw[:], cos_sb[:, tsl])
                nc.vector.tensor_mul(qss[:], swp[:], sin_sb[:, tsl])
                nc.vector.tensor_add(dst[:, tsl], qcs[:], qss[:])

        def emit_vtrans(st):
            b, vT, vtok = st["b"], st["vT"], st["vtok"]
            vbounce = vbounces[b % 2]
            nc.sync.dma_start(vbounce[0:D, :], vT[0:D, :])
            nc.sync.dma_start(vbounce[D + 1 : 2 * D + 1, :], vT[D : 2 * D, :])
            nc.sync.dma_start(vbounce[D : D + 1, :], ones_row[:])
            nc.sync.dma_start(vbounce[2 * D + 1 : 2 * D + 2, :], ones_row[:])
            nc.sync.dma_start_transpose(vtok[:, :, :], vbounce[:, :])

        def emit_qk_exp(st, qc, kt):
            qTt, kTt = st["qT"], st["kT"]
            qsl = slice(qc * QC, (qc + 1) * QC)
            ksl = slice(kt * P, (kt + 1) * P)
            pss = ps_s.tile([P, 2, QC], F32, tag="pss", name="pss_g")
            pexp = pexp_pool.tile([P, 2, QC], BF, tag="pexp", name="pexp_g")
            nc.tensor.matmul(
                pss[:, 0, :], kTt[0:D, ksl], qTt[0:D, qsl],
                start=True, stop=True, tile_position=(0, 0), skip_group_check=True,
            )
            nc.tensor.matmul(
                pss[:, 1, :], kTt[D : 2 * D, ksl], qTt[D : 2 * D, qsl],
                start=True, stop=True, tile_position=(64, 0), skip_group_check=True,
            )
            nc.scalar.activation(
                pexp[:], pss[:], mybir.ActivationFunctionType.Exp, scale=0.125
            )
            return pexp

        def emit_pv(st, kt, pexp, po0, po1):
            vtok = st["vtok"]
            nc.tensor.matmul(
                po0[:], vtok[:, kt, 0 : D + 1], pexp[:, 0, :],
                start=(kt == 0), stop=(kt == NKT - 1), skip_group_check=True,
            )
            nc.tensor.matmul(
                po1[:], vtok[:, kt, D + 1 : 2 * D + 2], pexp[:, 1, :],
                start=(kt == 0), stop=(kt == NKT - 1), skip_group_check=True,
            )

        def emit_norm(po0, po1):
            rs = small.tile([1, 2, QC], F32, tag="rs", name="rs")
            rr = small.tile([1, 2, QC], F32, tag="rr", name="rr")
            bc = small.tile([D, 2, QC], F32, tag="bc", name="bc")
            nc.vector.tensor_copy(rs[:, 0, :], po0[D : D + 1, :])
            nc.vector.tensor_copy(rs[:, 1, :], po1[D : D + 1, :])
            nc.vector.reciprocal_approx_fast(rr[:], rs[:])
            # Pool runs ONLY PartitionBroadcast ucode: any other op type on it
            # forces a multi-us LIBRARY_RELOAD that serializes the norm chain
            nc.gpsimd.partition_broadcast(bc[:], rr[:])
            onorm = onorm_pool.tile([P, QC], BF, tag="onorm", name="onorm")
            nc.vector.tensor_mul(onorm[0:D, :], po0[0:D, :], bc[:, 0, :])
            nc.vector.tensor_mul(onorm[D : 2 * D, :], po1[0:D, :], bc[:, 1, :])
            return onorm

        def emit_outproj_step(st, qc, onorm, fc):
            t0 = st["b"] * NSEQ
            py = ps_po.tile([P, QC], F32, tag="po", name="py")
            nc.tensor.matmul(
                py[:], wout_sb[:, fc, :], onorm[:], start=True, stop=True,
                skip_group_check=True,
            )
            yt = ytmp_pool.tile([P, QC], F32, tag="yt", name="yt")
            nc.vector.tensor_copy(yt[:], py[:])
            nc.sync.dma_start(
                yT[fc * P : (fc + 1) * P, t0 + qc * QC : t0 + (qc + 1) * QC],
                yt[:],
            )

        # ---- pipelined emission ----
        fillers = []

        def emit_filler(n=1):
            for _ in range(n):
                if fillers:
                    fillers.pop(0)()

        def queue_batch_work(st):
            # k first, then q chunk 0 (QK prefetch of the batch's first query
            # chunk only needs these), then v/vtrans, then remaining q
            for t4 in range(4):
                fillers.append(lambda st=st, t4=t4: emit_proj_chunk(st, 1, t4))
            fillers.append(lambda st=st: emit_proj_chunk(st, 0, 0))
            for t4 in range(4):
                fillers.append(lambda st=st, t4=t4: emit_proj_chunk(st, 2, t4))
            fillers.append(lambda st=st: emit_vtrans(st))
            for t4 in range(1, 4):
                fillers.append(lambda st=st, t4=t4: emit_proj_chunk(st, 0, t4))

        states = [None] * B
        states[0] = emit_load(0)
        sched = [(b, qc) for b in range(B) for qc in range(NQC)]
        pexps = {}
        # lead-in: k + first q chunk, then QK+exp starts streaming while the
        # v projection and the rest of q are still being emitted
        for t4 in range(4):
            emit_proj_chunk(states[0], 1, t4)
        emit_proj_chunk(states[0], 0, 0)
        for kt in range(3):
            pexps[kt] = emit_qk_exp(states[0], 0, kt)
        for t4 in range(4):
            emit_proj_chunk(states[0], 2, t4)
            pexps[3 + t4] = emit_qk_exp(states[0], 0, 3 + t4)
        emit_vtrans(states[0])
        for t4 in range(1, 4):
            emit_proj_chunk(states[0], 0, t4)
        states[1] = emit_load(1)
        queue_batch_work(states[1])

        for idx, (b, qc) in enumerate(sched):
            st = states[b]
            if qc == 0 and b + 2 < B:
                states[b + 2] = emit_load(b + 2)
            # po allocs come after the previous boundary's py allocs
            po0 = ps_po.tile([D + 1, QC], F32, tag="po", name="po0")
            po1 = ps_po.tile([D + 1, QC], F32, tag="po", name="po1")
            # main slot loop: kts PRE..15; PV catches up two-per-slot
            nv = 0  # next PV kt
            for kt in range(PRE, NKT):
                pexps[kt] = emit_qk_exp(st, qc, kt)
                take = 2 if kt < NKT - 3 else 1
                for _ in range(take):
                    if nv <= kt - 2:
                        emit_pv(st, nv, pexps.pop(nv), po0, po1)
                        nv += 1
            while nv < NKT:
                emit_pv(st, nv, pexps.pop(nv), po0, po1)
                nv += 1
            onorm = emit_norm(po0, po1)
            # boundary: outproj + fillers + next chunk's first PRE QK+exp
            nxt = sched[idx + 1] if idx + 1 < len(sched) else None
            if nxt is not None and nxt[1] == 0:
                # next chunk starts a new batch: its qT/kT writers must all be
                # emitted before that batch's QK reads (program order = exec
                # order per engine; no reordering happens at runtime)
                emit_filler(len(fillers))
                if nxt[0] + 1 < B and nxt[0] >= 1:
                    queue_batch_work(states[nxt[0] + 1])
            npre = 0

            def next_pre():
                nonlocal npre
                if nxt is not None and npre < PRE:
                    pexps[npre] = emit_qk_exp(states[nxt[0]], nxt[1], npre)
                    npre += 1

            next_pre()
            emit_filler()
            emit_outproj_step(st, qc, onorm, 0)
            emit_outproj_step(st, qc, onorm, 1)
            next_pre()
            emit_filler()
            emit_outproj_step(st, qc, onorm, 2)
            emit_outproj_step(st, qc, onorm, 3)
            next_pre()
            emit_filler()
            emit_outproj_step(st, qc, onorm, 4)
            emit_outproj_step(st, qc, onorm, 5)
            next_pre()
            emit_filler()
            emit_outproj_step(st, qc, onorm, 6)
            emit_outproj_step(st, qc, onorm, 7)
            for _ in range(PRE - npre):
                next_pre()
        emit_filler(len(fillers))

    nc.compile()
    return nc


def _host_inputs(x, cos, sin, Wq, Wkv, Wout):
    bf = ml_dtypes.bfloat16
    xT = np.ascontiguousarray(x.reshape(NTOK, C).T).astype(bf)
    cosT = cos.reshape(NSEQ, D).T.astype(np.float32)
    sinT = sin.reshape(NSEQ, D).T.astype(np.float32)
    sign = np.where(np.arange(D)[:, None] < D // 2, -1.0, 1.0).astype(np.float32)
    cos2 = np.ascontiguousarray(np.concatenate([cosT, cosT], 0))
    sin2s = np.ascontiguousarray(np.concatenate([sinT * sign, sinT * sign], 0))
    maps = []
    for core in range(8):
        c0 = core * P
        maps.append(
            {
                "xT": xT,
                "wq": np.ascontiguousarray(Wq[:, c0 : c0 + P]).astype(bf),
                "wk": np.ascontiguousarray(Wkv[:, c0 : c0 + P]).astype(bf),
                "wv": np.ascontiguousarray(Wkv[:, C + c0 : C + c0 + P]).astype(bf),
                "wout": np.ascontiguousarray(Wout[c0 : c0 + P, :]).astype(bf),
                "cos2": cos2,
                "sin2s": sin2s,
            }
        )
    return maps


_nc_cache = None


def _get_nc():
    global _nc_cache
    if _nc_cache is None:
        _nc_cache = _build()
    return _nc_cache


def kernel(x, cos, sin, Wq, Wkv, Wout, bout, _trace=False):
    x = np.asarray(x, dtype=np.float32)
    cos = np.asarray(cos, dtype=np.float32)
    sin = np.asarray(sin, dtype=np.float32)
    Wq = np.asarray(Wq, dtype=np.float32)
    Wkv = np.asarray(Wkv, dtype=np.float32)
    Wout = np.asarray(Wout, dtype=np.float32)
    bout = np.asarray(bout, dtype=np.float32)

    nc = _get_nc()
    in_maps = _host_inputs(x, cos, sin, Wq, Wkv, Wout)
    res = run_bass_kernel_spmd(nc, in_maps, list(range(8)), trace=_trace)

    y = np.zeros((C, NTOK), np.float32)
    for c in range(8):
        y += res.results[c]["yT"]
    out = y.T.reshape(B, NSEQ, C) + bout
    if _trace:
        return out, res
    return out


# revision 14
# speedup vs baseline: 1.0607x; 1.0607x over previous
"""Multi-head attention with RoPE (B=4, N=2048, C=1024, H=16, d=64) on 8
Trainium2 NeuronCores.

Sharding: tensor-parallel over heads — each core computes 2 of the 16 heads
(Wq/Wkv sharded column-wise, Wout row-wise). Each core returns a partial
yT = (out_h @ Wout_h).T over the full batch; the host sums the 8 partials.

Per-core kernel (all matmuls bf16, fp32 PSUM accumulation), software-pipelined
so ScalarE streams exp() continuously while the PE array stays dense:
  - Projections run W-stationary one 512-token chunk at a time (8 contraction
    matmuls into one dedicated PSUM bank pair); chunks are emitted as filler
    work at attention chunk boundaries. RoPE is applied on evacuation
    (rotate-half via partition-swapped SBUF copy, sign folded into the host
    sin table); all PSUM evacuations of projections run on VectorE so ScalarE
    does nothing but exp.
  - v is transposed token-major via DRAM bounce + XBAR DMA transpose with
    interleaved ones columns so PV (M=65) also yields softmax denominators.
  - Attention per 512-query chunk: S^T tiles = k.T@q, two heads per PE pass
    via row-group tile_position; exp on ScalarE from PSUM (scale=1/8 folded;
    |S/8| < 3 so no max subtraction); PV accumulates O^T + denominators.
  - The chunk boundary is software-pipelined: output projection of chunk i is
    interleaved with projection-chunk fillers and the first 7 QK+exp groups
    of chunk i+1, so neither PE nor ScalarE idles; PV of chunk i+1 catches up
    two-per-slot once its PSUM accumulators free.
"""

import numpy as np
import ml_dtypes
from contextlib import ExitStack

import concourse.bass as bass
import concourse.tile as tile
from concourse import bacc, mybir
from concourse.bass_utils import run_bass_kernel_spmd

P = 128
B, NSEQ, C = 4, 2048, 1024
H, D = 16, 64
NTOK = B * NSEQ
KO = C // P
QC = 512
NKT = NSEQ // P
NQC = NSEQ // QC
FC = C // P
VW = 160  # vtok row width: [v_h0 | 1 | v_h1 | 1 | pad] (32-multiple for XBAR)
BF = mybir.dt.bfloat16
F32 = mybir.dt.float32

PRE = 7  # QK+exp groups of chunk i+1 emitted during chunk i's boundary


def _build():
    nc = bacc.Bacc("TRN2", target_bir_lowering=False, debug=False)

    xT = nc.dram_tensor("xT", [C, NTOK], BF, kind="ExternalInput").ap()
    wq = nc.dram_tensor("wq", [C, P], BF, kind="ExternalInput").ap()
    wk = nc.dram_tensor("wk", [C, P], BF, kind="ExternalInput").ap()
    wv = nc.dram_tensor("wv", [C, P], BF, kind="ExternalInput").ap()
    wout = nc.dram_tensor("wout", [P, C], BF, kind="ExternalInput").ap()
    cos2 = nc.dram_tensor("cos2", [P, NSEQ], F32, kind="ExternalInput").ap()
    sin2s = nc.dram_tensor("sin2s", [P, NSEQ], F32, kind="ExternalInput").ap()
    yT = nc.dram_tensor("yT", [C, NTOK], F32, kind="ExternalOutput").ap()

    with ExitStack() as ctx:
        tc = ctx.enter_context(tile.TileContext(nc))
        consts = ctx.enter_context(tc.tile_pool(name="consts", bufs=1))
        xpool = ctx.enter_context(tc.tile_pool(name="xpool", bufs=2))
        qkpool = ctx.enter_context(tc.tile_pool(name="qkpool", bufs=2))
        vpool = ctx.enter_context(tc.tile_pool(name="vpool", bufs=2))
        rope = ctx.enter_context(tc.tile_pool(name="rope", bufs=2))
        pexp_pool = ctx.enter_context(tc.tile_pool(name="pexp", bufs=10))
        onorm_pool = ctx.enter_context(tc.tile_pool(name="onorm", bufs=3))
        ytmp_pool = ctx.enter_context(tc.tile_pool(name="ytmp", bufs=4))
        small = ctx.enter_context(tc.tile_pool(name="small", bufs=2))
        dram = ctx.enter_context(tc.tile_pool(name="dram", bufs=2, space="DRAM"))
        ps_s = ctx.enter_context(tc.tile_pool(name="ps_s", bufs=2, space="PSUM"))
        ps_po = ctx.enter_context(tc.tile_pool(name="ps_po", bufs=2, space="PSUM"))
        ps_pj = ctx.enter_context(tc.tile_pool(name="ps_pj", bufs=2, space="PSUM"))

        # ---- constants ----
        wq_sb = consts.tile([P, KO, P], BF, tag="wq")
        wk_sb = consts.tile([P, KO, P], BF, tag="wk")
        wv_sb = consts.tile([P, KO, P], BF, tag="wv")
        wout_sb = consts.tile([P, FC, P], BF, tag="wout")
        cos_sb = consts.tile([P, NSEQ], F32, tag="cos")
        sin_sb = consts.tile([P, NSEQ], F32, tag="sin")
        nc.sync.dma_start(wq_sb[:], wq.rearrange("(ko p) f -> p ko f", p=P))
        nc.sync.dma_start(wk_sb[:], wk.rearrange("(ko p) f -> p ko f", p=P))
        nc.sync.dma_start(wv_sb[:], wv.rearrange("(ko p) f -> p ko f", p=P))
        nc.sync.dma_start(wout_sb[:], wout.rearrange("r (fc f) -> r fc f", f=P))
        nc.sync.dma_start(cos_sb[:], cos2)
        nc.sync.dma_start(sin_sb[:], sin2s)
        ones_row = consts.tile([1, NSEQ], BF, tag="ones_row")
        nc.vector.memset(ones_row[:], 1.0)
        ones_blk = consts.tile([32, NSEQ], BF, tag="ones_blk")
        nc.vector.memset(ones_blk[:], 1.0)
        vbounces = []
        for i in range(2):
            vb = dram.tile([VW, NSEQ], BF, tag="vbounce", name=f"vb{i}")
            nc.sync.dma_start(vb[2 * D + 2 : VW, :], ones_blk[: VW - 2 * D - 2, :])
            vbounces.append(vb)

        def emit_load(b):
            t0 = b * NSEQ
            xb = xpool.tile([P, KO, NSEQ], BF, tag="xb", name="xb")
            xr = xT[:, t0 : t0 + NSEQ].rearrange("(ko p) t -> p ko t", p=P)
            # half-slab granular: first projection chunks wait on half the
            # batch's x (keeps 2KB/partition DMA lines for full throughput)
            for t2 in range(2):
                tsl = slice(t2 * (NSEQ // 2), (t2 + 1) * (NSEQ // 2))
                for ko in range(KO):
                    nc.sync.dma_start(xb[:, ko, tsl], xr[:, ko, tsl])
            qTt = qkpool.tile([P, NSEQ], BF, tag="qT", name="qT")
            kTt = qkpool.tile([P, NSEQ], BF, tag="kT", name="kT")
            vTt = qkpool.tile([P, NSEQ], BF, tag="vT", name="vT")
            vtok = vpool.tile([P, NKT, VW], BF, tag="vtok", name="vtok")
            return dict(xb=xb, qT=qTt, kT=kTt, vT=vTt, vtok=vtok, b=b)

        def emit_proj_chunk(st, f, t4):
            """One 512-token chunk of projection f (0=q, 1=k, 2=v):
            8 contraction matmuls into one PSUM bank + evacuation tail.
            All evacuations on VectorE (ScalarE is reserved for exp)."""
            w_sb, dst = [(wq_sb, st["qT"]), (wk_sb, st["kT"]), (wv_sb, st["vT"])][f]
            tsl = slice(t4 * QC, (t4 + 1) * QC)
            pj = ps_pj.tile([P, QC], F32, tag="pj", name="pj")
            for ko in range(KO):
                nc.tensor.matmul(
                    pj[:],
                    w_sb[:, ko, :],
                    st["xb"][:, ko, tsl],
                    start=(ko == 0),
                    stop=(ko == KO - 1),
                    skip_group_check=True,
                )
            if f == 2:
                nc.vector.tensor_copy(st["vT"][:, tsl], pj[:])
            else:
                raw = rope.tile([P, QC], F32, tag="raw", name="raw")
                swp = rope.tile([P, QC], F32, tag="swp", name="swp")
                qcs = rope.tile([P, QC], F32, tag="qcs", name="qcs")
                qss = rope.tile([P, QC], F32, tag="qss", name="qss")
                nc.vector.tensor_copy(raw[:], pj[:])
                for blk in range(4):
                    src = (blk ^ 1) * 32
                    nc.sync.dma_start(
                        swp[blk * 32 : blk * 32 + 32, :], raw[src : src + 32, :]
                    )
                nc.vector.tensor_mul(qcs[:], raw[:], cos_sb[:, tsl])
                nc.vector.tensor_mul(qss[:], swp[:], sin_sb[:, tsl])
                nc.vector.tensor_add(dst[:, tsl], qcs[:], qss[:])

        def emit_vtrans(st):
            b, vT, vtok = st["b"], st["vT"], st["vtok"]
            vbounce = vbounces[b % 2]
            nc.sync.dma_start(vbounce[0:D, :], vT[0:D, :])
            nc.sync.dma_start(vbounce[D + 1 : 2 * D + 1, :], vT[D : 2 * D, :])
            nc.sync.dma_start(vbounce[D : D + 1, :], ones_row[:])
            nc.sync.dma_start(vbounce[2 * D + 1 : 2 * D + 2, :], ones_row[:])
            nc.sync.dma_start_transpose(vtok[:, :, :], vbounce[:, :])

        def emit_qk_exp(st, qc, kt):
            qTt, kTt = st["qT"], st["kT"]
            qsl = slice(qc * QC, (qc + 1) * QC)
            ksl = slice(kt * P, (kt + 1) * P)
            pss = ps_s.tile([P, 2, QC], F32, tag="pss", name="pss_g")
            pexp = pexp_pool.tile([P, 2, QC], BF, tag="pexp", name="pexp_g")
            nc.tensor.matmul(
                pss[:, 0, :], kTt[0:D, ksl], qTt[0:D, qsl],
                start=True, stop=True, tile_position=(0, 0), skip_group_check=True,
            )
            nc.tensor.matmul(
                pss[:, 1, :], kTt[D : 2 * D, ksl], qTt[D : 2 * D, qsl],
                start=True, stop=True, tile_position=(64, 0), skip_group_check=True,
            )
            nc.scalar.activation(
                pexp[:], pss[:], mybir.ActivationFunctionType.Exp, scale=0.125
            )
            return pexp

        def emit_pv(st, kt, pexp, po0, po1):
            vtok = st["vtok"]
            nc.tensor.matmul(
                po0[:], vtok[:, kt, 0 : D + 1], pexp[:, 0, :],
                start=(kt == 0), stop=(kt == NKT - 1), skip_group_check=True,
            )
            nc.tensor.matmul(
                po1[:], vtok[:, kt, D + 1 : 2 * D + 2], pexp[:, 1, :],
                start=(kt == 0), stop=(kt == NKT - 1), skip_group_check=True,
            )

        def emit_norm(po0, po1):
            rs = small.tile([1, 2, QC], F32, tag="rs", name="rs")
            rr = small.tile([1, 2, QC], F32, tag="rr", name="rr")
            bc = small.tile([D, 2, QC], F32, tag="bc", name="bc")
            nc.vector.tensor_copy(rs[:, 0, :], po0[D : D + 1, :])
            nc.vector.tensor_copy(rs[:, 1, :], po1[D : D + 1, :])
            nc.vector.reciprocal_approx_fast(rr[:], rs[:])
            # Pool runs ONLY PartitionBroadcast ucode: any other op type on it
            # forces a multi-us LIBRARY_RELOAD that serializes the norm chain
            nc.gpsimd.partition_broadcast(bc[:], rr[:])
            onorm = onorm_pool.tile([P, QC], BF, tag="onorm", name="onorm")
            nc.vector.tensor_mul(onorm[0:D, :], po0[0:D, :], bc[:, 0, :])
            nc.vector.tensor_mul(onorm[D : 2 * D, :], po1[0:D, :], bc[:, 1, :])
            return onorm

        def emit_outproj_step(st, qc, onorm, fc):
            t0 = st["b"] * NSEQ
            py = ps_po.tile([P, QC], F32, tag="po", name="py")
            nc.tensor.matmul(
                py[:], wout_sb[:, fc, :], onorm[:], start=True, stop=True,
                skip_group_check=True,
            )
            yt = ytmp_pool.tile([P, QC], F32, tag="yt", name="yt")
            nc.vector.tensor_copy(yt[:], py[:])
            nc.sync.dma_start(
                yT[fc * P : (fc + 1) * P, t0 + qc * QC : t0 + (qc + 1) * QC],
                yt[:],
            )

        # ---- pipelined emission ----
        fillers = []

        def emit_filler(n=1):
            for _ in range(n):
                if fillers:
                    fillers.pop(0)()

        def queue_batch_work(st):
            # k first, then q chunk 0 (QK prefetch of the batch's first query
            # chunk only needs these), then v/vtrans, then remaining q
            for t4 in range(4):
                fillers.append(lambda st=st, t4=t4: emit_proj_chunk(st, 1, t4))
            fillers.append(lambda st=st: emit_proj_chunk(st, 0, 0))
            for t4 in range(4):
                fillers.append(lambda st=st, t4=t4: emit_proj_chunk(st, 2, t4))
            fillers.append(lambda st=st: emit_vtrans(st))
            for t4 in range(1, 4):
                fillers.append(lambda st=st, t4=t4: emit_proj_chunk(st, 0, t4))

        states = [None] * B
        states[0] = emit_load(0)
        sched = [(b, qc) for b in range(B) for qc in range(NQC)]
        pexps = {}
        # lead-in: k + first q chunk, then QK+exp starts streaming while the
        # v projection and the rest of q are still being emitted
        for t4 in range(4):
            emit_proj_chunk(states[0], 1, t4)
        emit_proj_chunk(states[0], 0, 0)
        for kt in range(3):
            pexps[kt] = emit_qk_exp(states[0], 0, kt)
        for t4 in range(4):
            emit_proj_chunk(states[0], 2, t4)
            pexps[3 + t4] = emit_qk_exp(states[0], 0, 3 + t4)
        emit_vtrans(states[0])
        for t4 in range(1, 4):
            emit_proj_chunk(states[0], 0, t4)
        states[1] = emit_load(1)
        queue_batch_work(states[1])

        for idx, (b, qc) in enumerate(sched):
            st = states[b]
            if qc == 0 and b + 2 < B:
                states[b + 2] = emit_load(b + 2)
            # po allocs come after the previous boundary's py allocs
            po0 = ps_po.tile([D + 1, QC], F32, tag="po", name="po0")
            po1 = ps_po.tile([D + 1, QC], F32, tag="po", name="po1")
            # main slot loop: kts PRE..15; PV catches up two-per-slot
            nv = 0  # next PV kt
            for kt in range(PRE, NKT):
                pexps[kt] = emit_qk_exp(st, qc, kt)
                take = 2 if kt < NKT - 3 else 1
                for _ in range(take):
                    if nv <= kt - 2:
                        emit_pv(st, nv, pexps.pop(nv), po0, po1)
                        nv += 1
            while nv < NKT:
                emit_pv(st, nv, pexps.pop(nv), po0, po1)
                nv += 1
            onorm = emit_norm(po0, po1)
            # boundary: outproj + fillers + next chunk's first PRE QK+exp
            nxt = sched[idx + 1] if idx + 1 < len(sched) else None
            if nxt is not None and nxt[1] == 0:
                # next chunk starts a new batch: its qT/kT writers must all be
                # emitted before that batch's QK reads (program order = exec
                # order per engine; no reordering happens at runtime)
                emit_filler(len(fillers))
                if nxt[0] + 1 < B and nxt[0] >= 1:
                    queue_batch_work(states[nxt[0] + 1])
            npre = 0

            def next_pre():
                nonlocal npre
                if nxt is not None and npre < PRE:
                    pexps[npre] = emit_qk_exp(states[nxt[0]], nxt[1], npre)
                    npre += 1

            next_pre()
            emit_filler()
            emit_outproj_step(st, qc, onorm, 0)
            emit_outproj_step(st, qc, onorm, 1)
            next_pre()
            emit_filler()
            emit_outproj_step(st, qc, onorm, 2)
            emit_outproj_step(st, qc, onorm, 3)
            next_pre()
            emit_filler()
            emit_outproj_step(st, qc, onorm, 4)
            emit_outproj_step(st, qc, onorm, 5)
            next_pre()
            emit_filler()
            emit_outproj_step(st, qc, onorm, 6)
            emit_outproj_step(st, qc, onorm, 7)
            for _ in range(PRE - npre):
                next_pre()
        emit_filler(len(fillers))

    nc.compile()
    return nc


def _host_inputs(x, cos, sin, Wq, Wkv, Wout):
    bf = ml_dtypes.bfloat16
    xT = np.ascontiguousarray(x.reshape(NTOK, C).T).astype(bf)
    cosT = cos.reshape(NSEQ, D).T.astype(np.float32)
    sinT = sin.reshape(NSEQ, D).T.astype(np.float32)
    sign = np.where(np.arange(D)[:, None] < D // 2, -1.0, 1.0).astype(np.float32)
    cos2 = np.ascontiguousarray(np.concatenate([cosT, cosT], 0))
    sin2s = np.ascontiguousarray(np.concatenate([sinT * sign, sinT * sign], 0))
    maps = []
    for core in range(8):
        c0 = core * P
        maps.append(
            {
                "xT": xT,
                "wq": np.ascontiguousarray(Wq[:, c0 : c0 + P]).astype(bf),
                "wk": np.ascontiguousarray(Wkv[:, c0 : c0 + P]).astype(bf),
                "wv": np.ascontiguousarray(Wkv[:, C + c0 : C + c0 + P]).astype(bf),
                "wout": np.ascontiguousarray(Wout[c0 : c0 + P, :]).astype(bf),
                "cos2": cos2,
                "sin2s": sin2s,
            }
        )
    return maps


_nc_cache = None


def _get_nc():
    global _nc_cache
    if _nc_cache is None:
        _nc_cache = _build()
    return _nc_cache


def kernel(x, cos, sin, Wq, Wkv, Wout, bout, _trace=False):
    x = np.asarray(x, dtype=np.float32)
    cos = np.asarray(cos, dtype=np.float32)
    sin = np.asarray(sin, dtype=np.float32)
    Wq = np.asarray(Wq, dtype=np.float32)
    Wkv = np.asarray(Wkv, dtype=np.float32)
    Wout = np.asarray(Wout, dtype=np.float32)
    bout = np.asarray(bout, dtype=np.float32)

    nc = _get_nc()
    in_maps = _host_inputs(x, cos, sin, Wq, Wkv, Wout)
    res = run_bass_kernel_spmd(nc, in_maps, list(range(8)), trace=_trace)

    y = np.zeros((C, NTOK), np.float32)
    for c in range(8):
        y += res.results[c]["yT"]
    out = y.T.reshape(B, NSEQ, C) + bout
    if _trace:
        return out, res
    return out
